# revision 1
# baseline (speedup 1.0000x reference)
"""Trainium2 Bass kernel for nn_HIPABlock_42752104465010.

Structure of the computation (B=4, C=C_out=256, H=W=256, 4 pyramid levels):
  1. Grid max-pool pyramid: only the finest level (8x8 grid of 32x32 cells)
     needs the full input read; coarser levels are 2x2 maxes of it.
  2. Tiny middle: L2 norms, top-k selection, LayerNorm + Linear on <=32
     tokens per sample, coords. (KB-scale compute.)
  3. Dense output is piecewise-constant on the 8x8 grid of 32x32 blocks
     (level 1 keeps all 4 cells, finer levels overwrite), so it is a
     [C_out, 8, 8] value map broadcast up by 32x32 per sample.

Device plan (8 NeuronCores): core c = 2*s + h handles sample s = c//2,
half h = c%2 (image rows 128h : 128h+128 — aligned to 32-px cell rows).
  Kernel A (pool):  x_half [256, 128, 256] -> pooled [256, 32]   (reads 32 MiB)
  Host:             top-k / LN / Linear / valmap  (numpy, ~KBs)
  Kernel B (paint): vm [256, 32] -> out_half [256, 128, 256]     (writes 32 MiB)
Total HBM traffic = 512 MiB across 8 cores == the memory roofline.
"""

import numpy as np

import concourse.tile as tile
from concourse import bacc, mybir
from concourse import bass_utils
from concourse._compat import get_trn_type

B = 4
C = 256
C_OUT = 256
H = 256
W = 256
NUM_LEVELS = 4
KEEP_RATIO = 0.3
MIN_KEEPS = 8
EPS = 1e-5

G = 2 ** (NUM_LEVELS - 1)          # 8  finest grid
CELL = H // G                      # 32 pixels per finest cell
HALF_ROWS = H // 2                 # 128
GR_HALF = G // 2                   # 4  finest grid rows per half

F32 = mybir.dt.float32

_CACHE = {}


def _build_pool_kernel():
    """x_half [C, 128, 256] -> pooled [C, 4*8] (max over each 32x32 cell)."""
    nc = bacc.Bacc(get_trn_type() or "TRN2", target_bir_lowering=False,
                   debug=False, num_devices=8)
    x_t = nc.dram_tensor("x", [C, HALF_ROWS, W], F32, kind="ExternalInput")
    p_t = nc.dram_tensor("pooled", [C, GR_HALF * G], F32, kind="ExternalOutput")
    x_ap = x_t.ap()
    p_ap = p_t.ap()

    with tile.TileContext(nc) as tc:
        with tc.tile_pool(name="big", bufs=3) as big, \
             tc.tile_pool(name="acc", bufs=2) as acc:
            for cb in range(C // 128):
                pooled_sb = acc.tile([128, GR_HALF * G], F32)
                for r in range(GR_HALF):
                    t = big.tile([128, CELL * W], F32)
                    tv = t[:].rearrange("p (r c) -> p r c", r=CELL)
                    nc.sync.dma_start(
                        tv, x_ap[cb * 128:(cb + 1) * 128,
                                 r * CELL:(r + 1) * CELL, :])
                    # fold the 32 rows down to 1 via in-place pairwise max
                    width = CELL * W // 2
                    while width >= W:
                        nc.vector.tensor_max(
                            t[:, 0:width], t[:, 0:width], t[:, width:2 * width])
                        width //= 2
                    # rowmax [128, 256] -> per-cell max [128, 8]
                    nc.vector.reduce_max(
                        pooled_sb[:, r * G:(r + 1) * G],
                        t[:, 0:W].rearrange("p (j c) -> p j c", j=G),
                        axis=mybir.AxisListType.X)
                nc.sync.dma_start(p_ap[cb * 128:(cb + 1) * 128, :], pooled_sb[:])
    nc.compile()
    return nc


def _build_paint_kernel():
    """vm [C_OUT, 4*8] -> out_half [C_OUT, 128, 256] (32x32 broadcast)."""
    nc = bacc.Bacc(get_trn_type() or "TRN2", target_bir_lowering=False,
                   debug=False, num_devices=8)
    vm_t = nc.dram_tensor("vm", [C_OUT, GR_HALF * G], F32, kind="ExternalInput")
    y_t = nc.dram_tensor("out", [C_OUT, HALF_ROWS, W], F32, kind="ExternalOutput")
    vm_ap = vm_t.ap()
    y_ap = y_t.ap()

    with tile.TileContext(nc) as tc:
        with tc.tile_pool(name="const", bufs=1) as cpool, \
             tc.tile_pool(name="vm", bufs=2) as vpool, \
             tc.tile_pool(name="pat", bufs=4) as ppool:
            ones = cpool.tile([128, CELL], F32)
            nc.vector.memset(ones[:], 1.0)
            for cb in range(C_OUT // 128):
                vm_sb = vpool.tile([128, GR_HALF * G], F32)
                nc.sync.dma_start(
                    vm_sb[:], vm_ap[cb * 128:(cb + 1) * 128, :])
                for r in range(GR_HALF):
                    pat = ppool.tile([128, W], F32)
                    for j in range(G):
                        nc.vector.tensor_scalar_mul(
                            pat[:, j * CELL:(j + 1) * CELL], ones[:],
                            vm_sb[:, r * G + j:r * G + j + 1])
                    # one DMA paints 32 identical rows from the 1-row pattern
                    nc.sync.dma_start(
                        y_ap[cb * 128:(cb + 1) * 128,
                             r * CELL:(r + 1) * CELL, :],
                        pat[:].unsqueeze(1).broadcast_to([128, CELL, W]))
    nc.compile()
    return nc


def _get_kernels():
    if "pool" not in _CACHE:
        _CACHE["pool"] = _build_pool_kernel()
        _CACHE["paint"] = _build_paint_kernel()
    return _CACHE["pool"], _CACHE["paint"]


def _level_coords(g):
    c = (np.arange(g, dtype=np.float32) + 0.5) / g
    gy, gx = np.meshgrid(c, c, indexing="ij")
    centers = np.stack([gx, gy], axis=-1).reshape(-1, 2)
    sizes = np.full((g * g, 2), 1.0 / g, dtype=np.float32)
    return np.concatenate([centers, sizes], axis=-1).astype(np.float32)


def _middle(pooled8, ln_g, ln_b, w, b):
    """pooled8 [C, 8, 8] (one sample) -> (sparse_rows, coords_rows, valmap).

    Exactly mirrors the reference's per-level top-k / LayerNorm / Linear /
    coarse-to-fine paint, but on the 8x8 value map instead of dense HxW.
    """
    pyr = [None] * NUM_LEVELS
    pyr[NUM_LEVELS - 1] = pooled8
    for l in range(NUM_LEVELS - 2, -1, -1):
        g = 2 ** l
        p = pyr[l + 1].reshape(C, g, 2, g, 2)
        pyr[l] = p.max(axis=(2, 4))

    per_level = [None] * NUM_LEVELS
    for level in range(NUM_LEVELS - 1, -1, -1):
        g = 2 ** level
        N = g * g
        flat = pyr[level].reshape(C, N).T                    # [N, C]
        l2 = np.linalg.norm(flat.astype(np.float32), axis=1)
        if level < NUM_LEVELS - 1:
            parent = pyr[level + 1][:, ::2, ::2].reshape(C, N)
            pl2 = np.linalg.norm(parent, axis=0)
            imp = np.abs(l2 - pl2)
        else:
            imp = l2
        k = min(max(MIN_KEEPS, int(N * KEEP_RATIO)), N)
        idx = np.argsort(-imp, kind="stable")[:k]            # top_k order
        kf = flat[idx].astype(np.float32)                    # [k, C]
        mu = kf.mean(-1, keepdims=True)
        var = ((kf - mu) ** 2).mean(-1, keepdims=True)
        kfn = (kf - mu) / np.sqrt(var + EPS) * ln_g + ln_b
        kp = (kfn @ w + b).astype(np.float32)                # [k, C_OUT]
        kc = _level_coords(g)[idx]                           # [k, 4]
        per_level[level] = (kp, idx, kc)

    sparse_rows = np.concatenate([p[0] for p in per_level], axis=0)
    coords_rows = np.concatenate([p[2] for p in per_level], axis=0)

    valmap = np.zeros((C_OUT, G, G), np.float32)
    for level in range(NUM_LEVELS):
        kp, idx, _ = per_level[level]
        g = 2 ** level
        s = G // g
        for row, n in zip(kp, idx):
            y, x = divmod(int(n), g)
            valmap[:, y * s:(y + 1) * s, x * s:(x + 1) * s] = row[:, None, None]
    return sparse_rows, coords_rows, valmap


def kernel(x, ln_g, ln_b, w, b):
    x = np.asarray(x, np.float32)
    ln_g = np.asarray(ln_g, np.float32)
    ln_b = np.asarray(ln_b, np.float32)
    w = np.asarray(w, np.float32)
    b = np.asarray(b, np.float32)

    nc_pool, nc_paint = _get_kernels()

    # --- device pass 1: grid max-pool, data-parallel over (sample, half) ---
    in_maps = [
        {"x": np.ascontiguousarray(x[c // 2, :, (c % 2) * HALF_ROWS:(c % 2 + 1) * HALF_ROWS, :])}
        for c in range(8)
    ]
    r1 = bass_utils.run_bass_kernel_spmd(nc_pool, in_maps, core_ids=list(range(8)))
    pooled8 = np.empty((B, C, G, G), np.float32)
    for c in range(8):
        s, h = c // 2, c % 2
        pooled8[s, :, h * GR_HALF:(h + 1) * GR_HALF, :] = \
            r1.results[c]["pooled"].reshape(C, GR_HALF, G)

    # --- host middle: top-k / LayerNorm / Linear / coords / value map ---
    sparse_seq = np.empty((B, 32, C_OUT), np.float32)
    all_coords = np.empty((B, 32, 4), np.float32)
    valmaps = np.empty((B, C_OUT, G, G), np.float32)
    for s in range(B):
        sparse_seq[s], all_coords[s], valmaps[s] = _middle(
            pooled8[s], ln_g, ln_b, w, b)

    # --- device pass 2: broadcast-paint the dense output ---
    in_maps2 = [
        {"vm": np.ascontiguousarray(
            valmaps[c // 2, :, (c % 2) * GR_HALF:(c % 2 + 1) * GR_HALF, :]
        ).reshape(C_OUT, GR_HALF * G)}
        for c in range(8)
    ]
    r2 = bass_utils.run_bass_kernel_spmd(nc_paint, in_maps2, core_ids=list(range(8)))
    out = np.empty((B, C_OUT, H, W), np.float32)
    for c in range(8):
        s, h = c // 2, c % 2
        out[s, :, h * HALF_ROWS:(h + 1) * HALF_ROWS, :] = r2.results[c]["out"]

    sparsity = np.float32(sparse_seq.shape[1] / (H * W))
    return out, sparse_seq, all_coords, sparsity


# revision 2
# speedup vs baseline: 1.0429x; 1.0429x over previous
"""Trainium2 Bass kernel for nn_HIPABlock_42752104465010.

Structure of the computation (B=4, C=C_out=256, H=W=256, 4 pyramid levels):
  1. Grid max-pool pyramid: only the finest level (8x8 grid of 32x32 cells)
     needs the full input read; coarser levels are 2x2 maxes of it.
  2. Tiny middle: L2 norms, top-k selection, LayerNorm + Linear on <=32
     tokens per sample, coords. (KB-scale compute.)
  3. Dense output is piecewise-constant on the 8x8 grid of 32x32 blocks
     (level 1 keeps all 4 cells, finer levels overwrite), so it is a
     [C_out, 8, 8] value map broadcast up by 32x32 per sample.

Device plan (8 NeuronCores): core c = 2*s + h handles sample s = c//2,
half h = c%2 (image rows 128h : 128h+128 — aligned to 32-px cell rows).
  Kernel A (pool):  x_half [256, 128, 256] -> pooled [256, 32]   (reads 32 MiB)
  Host:             top-k / LN / Linear / valmap  (numpy, ~KBs)
  Kernel B (paint): vm [256, 32] -> out_half [256, 128, 256]     (writes 32 MiB)
Total HBM traffic = 512 MiB across 8 cores == the memory roofline.
"""

import numpy as np

import concourse.tile as tile
from concourse import bacc, mybir
from concourse import bass_utils
from concourse._compat import get_trn_type

B = 4
C = 256
C_OUT = 256
H = 256
W = 256
NUM_LEVELS = 4
KEEP_RATIO = 0.3
MIN_KEEPS = 8
EPS = 1e-5

G = 2 ** (NUM_LEVELS - 1)          # 8  finest grid
CELL = H // G                      # 32 pixels per finest cell
HALF_ROWS = H // 2                 # 128
GR_HALF = G // 2                   # 4  finest grid rows per half

F32 = mybir.dt.float32

_CACHE = {}


def _build_pool_kernel(sub=8, bufs=8):
    """x_half [C, 128, 256] -> pooled [C, 4*8] (max over each 32x32 cell).

    Each (channel-block, grid-row) is streamed as `sub` row-chunks; every
    chunk is reduced to its 8 per-cell partial maxes as soon as it lands
    (one 4D-AP reduce_max), then one tiny strided reduce combines the
    partials. This keeps the DVE tail after the last DMA to ~1 us.
    """
    nc = bacc.Bacc(get_trn_type() or "TRN2", target_bir_lowering=False,
                   debug=False, num_devices=8)
    x_t = nc.dram_tensor("x", [C, HALF_ROWS, W], F32, kind="ExternalInput")
    p_t = nc.dram_tensor("pooled", [C, GR_HALF * G], F32, kind="ExternalOutput")
    x_ap = x_t.ap()
    p_ap = p_t.ap()
    rows = CELL // sub

    with tile.TileContext(nc) as tc:
        with tc.tile_pool(name="big", bufs=bufs) as big, \
             tc.tile_pool(name="part", bufs=2) as part, \
             tc.tile_pool(name="acc", bufs=2) as acc:
            for cb in range(C // 128):
                pooled_sb = acc.tile([128, GR_HALF * G], F32)
                for r in range(GR_HALF):
                    partials = part.tile([128, sub * G], F32)
                    for k in range(sub):
                        t = big.tile([128, rows * W], F32)
                        nc.sync.dma_start(
                            t[:].rearrange("p (r c) -> p r c", r=rows),
                            x_ap[cb * 128:(cb + 1) * 128,
                                 r * CELL + k * rows: r * CELL + (k + 1) * rows, :])
                        v = t[:].rearrange("p (r j c) -> p j r c", r=rows, j=G)
                        nc.vector.reduce_max(partials[:, k * G:(k + 1) * G], v,
                                             axis=mybir.AxisListType.XY)
                    pv = partials[:].rearrange("p (k j) -> p j k", k=sub)
                    nc.vector.reduce_max(pooled_sb[:, r * G:(r + 1) * G], pv,
                                         axis=mybir.AxisListType.X)
                nc.sync.dma_start(p_ap[cb * 128:(cb + 1) * 128, :], pooled_sb[:])
    nc.compile()
    return nc


def _build_paint_kernel():
    """vm [C_OUT, 4*8] -> out_half [C_OUT, 128, 256] (32x32 broadcast)."""
    nc = bacc.Bacc(get_trn_type() or "TRN2", target_bir_lowering=False,
                   debug=False, num_devices=8)
    vm_t = nc.dram_tensor("vm", [C_OUT, GR_HALF * G], F32, kind="ExternalInput")
    y_t = nc.dram_tensor("out", [C_OUT, HALF_ROWS, W], F32, kind="ExternalOutput")
    vm_ap = vm_t.ap()
    y_ap = y_t.ap()

    with tile.TileContext(nc) as tc:
        with tc.tile_pool(name="const", bufs=1) as cpool, \
             tc.tile_pool(name="vm", bufs=2) as vpool, \
             tc.tile_pool(name="pat", bufs=4) as ppool:
            ones = cpool.tile([128, CELL], F32)
            nc.vector.memset(ones[:], 1.0)
            for cb in range(C_OUT // 128):
                vm_sb = vpool.tile([128, GR_HALF * G], F32)
                nc.sync.dma_start(
                    vm_sb[:], vm_ap[cb * 128:(cb + 1) * 128, :])
                for r in range(GR_HALF):
                    pat = ppool.tile([128, W], F32)
                    for j in range(G):
                        nc.vector.tensor_scalar_mul(
                            pat[:, j * CELL:(j + 1) * CELL], ones[:],
                            vm_sb[:, r * G + j:r * G + j + 1])
                    # one DMA paints 32 identical rows from the 1-row pattern
                    nc.sync.dma_start(
                        y_ap[cb * 128:(cb + 1) * 128,
                             r * CELL:(r + 1) * CELL, :],
                        pat[:].unsqueeze(1).broadcast_to([128, CELL, W]))
    nc.compile()
    return nc


def _get_kernels():
    if "pool" not in _CACHE:
        _CACHE["pool"] = _build_pool_kernel()
        _CACHE["paint"] = _build_paint_kernel()
    return _CACHE["pool"], _CACHE["paint"]


def _level_coords(g):
    c = (np.arange(g, dtype=np.float32) + 0.5) / g
    gy, gx = np.meshgrid(c, c, indexing="ij")
    centers = np.stack([gx, gy], axis=-1).reshape(-1, 2)
    sizes = np.full((g * g, 2), 1.0 / g, dtype=np.float32)
    return np.concatenate([centers, sizes], axis=-1).astype(np.float32)


def _middle(pooled8, ln_g, ln_b, w, b):
    """pooled8 [C, 8, 8] (one sample) -> (sparse_rows, coords_rows, valmap).

    Exactly mirrors the reference's per-level top-k / LayerNorm / Linear /
    coarse-to-fine paint, but on the 8x8 value map instead of dense HxW.
    """
    pyr = [None] * NUM_LEVELS
    pyr[NUM_LEVELS - 1] = pooled8
    for l in range(NUM_LEVELS - 2, -1, -1):
        g = 2 ** l
        p = pyr[l + 1].reshape(C, g, 2, g, 2)
        pyr[l] = p.max(axis=(2, 4))

    per_level = [None] * NUM_LEVELS
    for level in range(NUM_LEVELS - 1, -1, -1):
        g = 2 ** level
        N = g * g
        flat = pyr[level].reshape(C, N).T                    # [N, C]
        l2 = np.linalg.norm(flat.astype(np.float32), axis=1)
        if level < NUM_LEVELS - 1:
            parent = pyr[level + 1][:, ::2, ::2].reshape(C, N)
            pl2 = np.linalg.norm(parent, axis=0)
            imp = np.abs(l2 - pl2)
        else:
            imp = l2
        k = min(max(MIN_KEEPS, int(N * KEEP_RATIO)), N)
        idx = np.argsort(-imp, kind="stable")[:k]            # top_k order
        kf = flat[idx].astype(np.float32)                    # [k, C]
        mu = kf.mean(-1, keepdims=True)
        var = ((kf - mu) ** 2).mean(-1, keepdims=True)
        kfn = (kf - mu) / np.sqrt(var + EPS) * ln_g + ln_b
        kp = (kfn @ w + b).astype(np.float32)                # [k, C_OUT]
        kc = _level_coords(g)[idx]                           # [k, 4]
        per_level[level] = (kp, idx, kc)

    sparse_rows = np.concatenate([p[0] for p in per_level], axis=0)
    coords_rows = np.concatenate([p[2] for p in per_level], axis=0)

    valmap = np.zeros((C_OUT, G, G), np.float32)
    for level in range(NUM_LEVELS):
        kp, idx, _ = per_level[level]
        g = 2 ** level
        s = G // g
        for row, n in zip(kp, idx):
            y, x = divmod(int(n), g)
            valmap[:, y * s:(y + 1) * s, x * s:(x + 1) * s] = row[:, None, None]
    return sparse_rows, coords_rows, valmap


def kernel(x, ln_g, ln_b, w, b):
    x = np.asarray(x, np.float32)
    ln_g = np.asarray(ln_g, np.float32)
    ln_b = np.asarray(ln_b, np.float32)
    w = np.asarray(w, np.float32)
    b = np.asarray(b, np.float32)

    nc_pool, nc_paint = _get_kernels()

    # --- device pass 1: grid max-pool, data-parallel over (sample, half) ---
    in_maps = [
        {"x": np.ascontiguousarray(x[c // 2, :, (c % 2) * HALF_ROWS:(c % 2 + 1) * HALF_ROWS, :])}
        for c in range(8)
    ]
    r1 = bass_utils.run_bass_kernel_spmd(nc_pool, in_maps, core_ids=list(range(8)))
    pooled8 = np.empty((B, C, G, G), np.float32)
    for c in range(8):
        s, h = c // 2, c % 2
        pooled8[s, :, h * GR_HALF:(h + 1) * GR_HALF, :] = \
            r1.results[c]["pooled"].reshape(C, GR_HALF, G)

    # --- host middle: top-k / LayerNorm / Linear / coords / value map ---
    sparse_seq = np.empty((B, 32, C_OUT), np.float32)
    all_coords = np.empty((B, 32, 4), np.float32)
    valmaps = np.empty((B, C_OUT, G, G), np.float32)
    for s in range(B):
        sparse_seq[s], all_coords[s], valmaps[s] = _middle(
            pooled8[s], ln_g, ln_b, w, b)

    # --- device pass 2: broadcast-paint the dense output ---
    in_maps2 = [
        {"vm": np.ascontiguousarray(
            valmaps[c // 2, :, (c % 2) * GR_HALF:(c % 2 + 1) * GR_HALF, :]
        ).reshape(C_OUT, GR_HALF * G)}
        for c in range(8)
    ]
    r2 = bass_utils.run_bass_kernel_spmd(nc_paint, in_maps2, core_ids=list(range(8)))
    out = np.empty((B, C_OUT, H, W), np.float32)
    for c in range(8):
        s, h = c // 2, c % 2
        out[s, :, h * HALF_ROWS:(h + 1) * HALF_ROWS, :] = r2.results[c]["out"]

    sparsity = np.float32(sparse_seq.shape[1] / (H * W))
    return out, sparse_seq, all_coords, sparsity


# revision 3
# speedup vs baseline: 1.0430x; 1.0000x over previous
"""Trainium2 Bass kernel for nn_HIPABlock_42752104465010.

Structure of the computation (B=4, C=C_out=256, H=W=256, 4 pyramid levels):
  1. Grid max-pool pyramid: only the finest level (8x8 grid of 32x32 cells)
     needs the full input read; coarser levels are 2x2 maxes of it.
  2. Tiny middle: L2 norms, top-k selection, LayerNorm + Linear on <=32
     tokens per sample, coords. (KB-scale compute.)
  3. Dense output is piecewise-constant on the 8x8 grid of 32x32 blocks
     (level 1 keeps all 4 cells, finer levels overwrite), so it is a
     [C_out, 8, 8] value map broadcast up by 32x32 per sample.

Device plan (8 NeuronCores): core c = 2*s + h handles sample s = c//2 and
channel half h = c%2 (channels 128h : 128h+128, full image height). The
per-core slices of x/out are contiguous, so host-side sharding is zero-copy.
  Kernel A (pool):  x_half [128, 256, 256] -> pooled [128, 64]   (reads 32 MiB)
  Host:             top-k / LN / Linear / valmap  (numpy, ~KBs)
  Kernel B (paint): vm [128, 64] -> out_half [128, 256, 256]     (writes 32 MiB)
Total HBM traffic = 512 MiB across 8 cores == the memory roofline.
"""

import numpy as np

import concourse.tile as tile
from concourse import bacc, mybir
from concourse import bass_utils
from concourse._compat import get_trn_type

B = 4
C = 256
C_OUT = 256
H = 256
W = 256
NUM_LEVELS = 4
KEEP_RATIO = 0.3
MIN_KEEPS = 8
EPS = 1e-5

G = 2 ** (NUM_LEVELS - 1)          # 8  finest grid
CELL = H // G                      # 32 pixels per finest cell
CH = C // 2                        # 128 channels per core

F32 = mybir.dt.float32

_CACHE = {}


def _build_pool_kernel(sub=8, bufs=8):
    """x_half [128, 256, 256] -> pooled [128, 8*8] (max over 32x32 cells).

    Each grid-row is streamed as `sub` row-chunks; every chunk is reduced
    to its 8 per-cell partial maxes as soon as it lands (one 4D-AP
    reduce_max), then one tiny strided reduce combines the partials. This
    keeps the DVE tail after the last DMA to ~1 us.
    """
    nc = bacc.Bacc(get_trn_type() or "TRN2", target_bir_lowering=False,
                   debug=False, num_devices=8)
    x_ap = nc.dram_tensor("x", [CH, H, W], F32, kind="ExternalInput").ap()
    p_ap = nc.dram_tensor("pooled", [CH, G * G], F32, kind="ExternalOutput").ap()
    rows = CELL // sub

    with tile.TileContext(nc) as tc:
        with tc.tile_pool(name="big", bufs=bufs) as big, \
             tc.tile_pool(name="part", bufs=2) as part, \
             tc.tile_pool(name="acc", bufs=1) as acc:
            pooled_sb = acc.tile([CH, G * G], F32)
            for r in range(G):
                partials = part.tile([CH, sub * G], F32)
                for k in range(sub):
                    t = big.tile([CH, rows * W], F32)
                    nc.sync.dma_start(
                        t[:].rearrange("p (r c) -> p r c", r=rows),
                        x_ap[:, r * CELL + k * rows: r * CELL + (k + 1) * rows, :])
                    v = t[:].rearrange("p (r j c) -> p j r c", r=rows, j=G)
                    nc.vector.reduce_max(partials[:, k * G:(k + 1) * G], v,
                                         axis=mybir.AxisListType.XY)
                pv = partials[:].rearrange("p (k j) -> p j k", k=sub)
                nc.vector.reduce_max(pooled_sb[:, r * G:(r + 1) * G], pv,
                                     axis=mybir.AxisListType.X)
            nc.sync.dma_start(p_ap[:, :], pooled_sb[:])
    nc.compile()
    return nc


def _build_paint_kernel():
    """vm [128, 8*8] -> out_half [128, 256, 256] (32x32 broadcast).

    Per grid-row: fill a one-row pattern via per-partition-scalar
    broadcasts, then one stride-0-source DMA writes all 32 identical rows.
    """
    nc = bacc.Bacc(get_trn_type() or "TRN2", target_bir_lowering=False,
                   debug=False, num_devices=8)
    vm_ap = nc.dram_tensor("vm", [CH, G * G], F32, kind="ExternalInput").ap()
    y_ap = nc.dram_tensor("out", [CH, H, W], F32, kind="ExternalOutput").ap()

    with tile.TileContext(nc) as tc:
        with tc.tile_pool(name="const", bufs=1) as cpool, \
             tc.tile_pool(name="vm", bufs=1) as vpool, \
             tc.tile_pool(name="pat", bufs=4) as ppool:
            ones = cpool.tile([CH, CELL], F32)
            nc.vector.memset(ones[:], 1.0)
            vm_sb = vpool.tile([CH, G * G], F32)
            nc.sync.dma_start(vm_sb[:], vm_ap[:, :])
            for r in range(G):
                pat = ppool.tile([CH, W], F32)
                for j in range(G):
                    nc.vector.tensor_scalar_mul(
                        pat[:, j * CELL:(j + 1) * CELL], ones[:],
                        vm_sb[:, r * G + j:r * G + j + 1])
                nc.sync.dma_start(
                    y_ap[:, r * CELL:(r + 1) * CELL, :],
                    pat[:].unsqueeze(1).broadcast_to([CH, CELL, W]))
    nc.compile()
    return nc


def _get_kernels():
    if "pool" not in _CACHE:
        _CACHE["pool"] = _build_pool_kernel()
        _CACHE["paint"] = _build_paint_kernel()
    return _CACHE["pool"], _CACHE["paint"]


def _level_coords(g):
    c = (np.arange(g, dtype=np.float32) + 0.5) / g
    gy, gx = np.meshgrid(c, c, indexing="ij")
    centers = np.stack([gx, gy], axis=-1).reshape(-1, 2)
    sizes = np.full((g * g, 2), 1.0 / g, dtype=np.float32)
    return np.concatenate([centers, sizes], axis=-1).astype(np.float32)


def _middle(pooled8, ln_g, ln_b, w, b):
    """pooled8 [C, 8, 8] (one sample) -> (sparse_rows, coords_rows, valmap).

    Exactly mirrors the reference's per-level top-k / LayerNorm / Linear /
    coarse-to-fine paint, but on the 8x8 value map instead of dense HxW.
    """
    pyr = [None] * NUM_LEVELS
    pyr[NUM_LEVELS - 1] = pooled8
    for l in range(NUM_LEVELS - 2, -1, -1):
        g = 2 ** l
        p = pyr[l + 1].reshape(C, g, 2, g, 2)
        pyr[l] = p.max(axis=(2, 4))

    per_level = [None] * NUM_LEVELS
    for level in range(NUM_LEVELS - 1, -1, -1):
        g = 2 ** level
        N = g * g
        flat = pyr[level].reshape(C, N).T                    # [N, C]
        l2 = np.linalg.norm(flat.astype(np.float32), axis=1)
        if level < NUM_LEVELS - 1:
            parent = pyr[level + 1][:, ::2, ::2].reshape(C, N)
            pl2 = np.linalg.norm(parent, axis=0)
            imp = np.abs(l2 - pl2)
        else:
            imp = l2
        k = min(max(MIN_KEEPS, int(N * KEEP_RATIO)), N)
        idx = np.argsort(-imp, kind="stable")[:k]            # top_k order
        kf = flat[idx].astype(np.float32)                    # [k, C]
        mu = kf.mean(-1, keepdims=True)
        var = ((kf - mu) ** 2).mean(-1, keepdims=True)
        kfn = (kf - mu) / np.sqrt(var + EPS) * ln_g + ln_b
        kp = (kfn @ w + b).astype(np.float32)                # [k, C_OUT]
        kc = _level_coords(g)[idx]                           # [k, 4]
        per_level[level] = (kp, idx, kc)

    sparse_rows = np.concatenate([p[0] for p in per_level], axis=0)
    coords_rows = np.concatenate([p[2] for p in per_level], axis=0)

    valmap = np.zeros((C_OUT, G, G), np.float32)
    for level in range(NUM_LEVELS):
        kp, idx, _ = per_level[level]
        g = 2 ** level
        s = G // g
        for row, n in zip(kp, idx):
            y, x = divmod(int(n), g)
            valmap[:, y * s:(y + 1) * s, x * s:(x + 1) * s] = row[:, None, None]
    return sparse_rows, coords_rows, valmap


def kernel(x, ln_g, ln_b, w, b):
    x = np.ascontiguousarray(np.asarray(x, np.float32))
    ln_g = np.asarray(ln_g, np.float32)
    ln_b = np.asarray(ln_b, np.float32)
    w = np.asarray(w, np.float32)
    b = np.asarray(b, np.float32)

    nc_pool, nc_paint = _get_kernels()

    # --- device pass 1: grid max-pool, sharded (sample, channel-half) ---
    # x[s, h*128:(h+1)*128] is a contiguous view — zero-copy sharding.
    in_maps = [
        {"x": x[c // 2, (c % 2) * CH:(c % 2 + 1) * CH, :, :]}
        for c in range(8)
    ]
    r1 = bass_utils.run_bass_kernel_spmd(nc_pool, in_maps, core_ids=list(range(8)))
    pooled8 = np.empty((B, C, G, G), np.float32)
    for c in range(8):
        s, h = c // 2, c % 2
        pooled8[s, h * CH:(h + 1) * CH] = r1.results[c]["pooled"].reshape(CH, G, G)

    # --- host middle: top-k / LayerNorm / Linear / coords / value map ---
    sparse_seq = np.empty((B, 32, C_OUT), np.float32)
    all_coords = np.empty((B, 32, 4), np.float32)
    valmaps = np.empty((B, C_OUT, G, G), np.float32)
    for s in range(B):
        sparse_seq[s], all_coords[s], valmaps[s] = _middle(
            pooled8[s], ln_g, ln_b, w, b)

    # --- device pass 2: broadcast-paint the dense output ---
    in_maps2 = [
        {"vm": valmaps[c // 2, (c % 2) * CH:(c % 2 + 1) * CH].reshape(CH, G * G)}
        for c in range(8)
    ]
    r2 = bass_utils.run_bass_kernel_spmd(nc_paint, in_maps2, core_ids=list(range(8)))
    out = np.empty((B, C_OUT, H, W), np.float32)
    for c in range(8):
        s, h = c // 2, c % 2
        out[s, h * CH:(h + 1) * CH] = r2.results[c]["out"]

    sparsity = np.float32(sparse_seq.shape[1] / (H * W))
    return out, sparse_seq, all_coords, sparsity


# revision 4
# speedup vs baseline: 1.0457x; 1.0026x over previous
"""Trainium2 Bass kernel for nn_HIPABlock_42752104465010.

Structure of the computation (B=4, C=C_out=256, H=W=256, 4 pyramid levels):
  1. Grid max-pool pyramid: only the finest level (8x8 grid of 32x32 cells)
     needs the full input read; coarser levels are 2x2 maxes of it.
  2. Tiny middle: L2 norms, top-k selection, LayerNorm + Linear on <=32
     tokens per sample, coords. (KB-scale compute.)
  3. Dense output is piecewise-constant on the 8x8 grid of 32x32 blocks
     (level 1 keeps all 4 cells, finer levels overwrite), so it is a
     [C_out, 8, 8] value map broadcast up by 32x32 per sample.

Device plan (8 NeuronCores): core c = 2*s + h handles sample s = c//2 and
channel half h = c%2 (channels 128h : 128h+128, full image height). The
per-core slices of x/out are contiguous, so host-side sharding is zero-copy.
  Kernel A (pool):  x_half [128, 256, 256] -> pooled [128, 64]   (reads 32 MiB)
  Host:             top-k / LN / Linear / valmap  (numpy, ~KBs)
  Kernel B (paint): vm [128, 64] -> out_half [128, 256, 256]     (writes 32 MiB)
Total HBM traffic = 512 MiB across 8 cores == the memory roofline.
"""

import numpy as np

import concourse.tile as tile
from concourse import bacc, mybir
from concourse import bass_utils
from concourse._compat import get_trn_type

B = 4
C = 256
C_OUT = 256
H = 256
W = 256
NUM_LEVELS = 4
KEEP_RATIO = 0.3
MIN_KEEPS = 8
EPS = 1e-5

G = 2 ** (NUM_LEVELS - 1)          # 8  finest grid
CELL = H // G                      # 32 pixels per finest cell
CH = C // 2                        # 128 channels per core

F32 = mybir.dt.float32

_CACHE = {}


def _build_pool_kernel(sub=8, bufs=8):
    """x_half [128, 256, 256] -> pooled [128, 8*8] (max over 32x32 cells).

    Each grid-row is streamed as `sub` row-chunks; every chunk is reduced
    to its 8 per-cell partial maxes as soon as it lands (one 4D-AP
    reduce_max), then one tiny strided reduce combines the partials. This
    keeps the DVE tail after the last DMA to ~1 us.
    """
    nc = bacc.Bacc(get_trn_type() or "TRN2", target_bir_lowering=False,
                   debug=False, num_devices=8)
    x_ap = nc.dram_tensor("x", [CH, H, W], F32, kind="ExternalInput").ap()
    p_ap = nc.dram_tensor("pooled", [CH, G * G], F32, kind="ExternalOutput").ap()
    rows = CELL // sub

    with tile.TileContext(nc) as tc:
        with tc.tile_pool(name="big", bufs=bufs) as big, \
             tc.tile_pool(name="part", bufs=2) as part, \
             tc.tile_pool(name="acc", bufs=1) as acc:
            pooled_sb = acc.tile([CH, G * G], F32)
            for r in range(G):
                partials = part.tile([CH, sub * G], F32)
                for k in range(sub):
                    t = big.tile([CH, rows * W], F32)
                    nc.sync.dma_start(
                        t[:].rearrange("p (r c) -> p r c", r=rows),
                        x_ap[:, r * CELL + k * rows: r * CELL + (k + 1) * rows, :])
                    v = t[:].rearrange("p (r j c) -> p j r c", r=rows, j=G)
                    nc.vector.reduce_max(partials[:, k * G:(k + 1) * G], v,
                                         axis=mybir.AxisListType.XY)
                pv = partials[:].rearrange("p (k j) -> p j k", k=sub)
                nc.vector.reduce_max(pooled_sb[:, r * G:(r + 1) * G], pv,
                                     axis=mybir.AxisListType.X)
            nc.sync.dma_start(p_ap[:, :], pooled_sb[:])
    nc.compile()
    return nc


def _build_paint_kernel():
    """vm [128, 8*8] -> out_half [128, 256, 256] (32x32 broadcast).

    Per grid-row: one stride-0 broadcast tensor_copy expands the 8 cell
    values into a one-row pattern, then one stride-0-source DMA writes all
    32 identical rows.
    """
    nc = bacc.Bacc(get_trn_type() or "TRN2", target_bir_lowering=False,
                   debug=False, num_devices=8)
    vm_ap = nc.dram_tensor("vm", [CH, G * G], F32, kind="ExternalInput").ap()
    y_ap = nc.dram_tensor("out", [CH, H, W], F32, kind="ExternalOutput").ap()

    with tile.TileContext(nc) as tc:
        with tc.tile_pool(name="vm", bufs=1) as vpool, \
             tc.tile_pool(name="pat", bufs=4) as ppool:
            vm_sb = vpool.tile([CH, G * G], F32)
            nc.sync.dma_start(vm_sb[:], vm_ap[:, :])
            for r in range(G):
                pat = ppool.tile([CH, W], F32)
                nc.vector.tensor_copy(
                    pat[:].rearrange("p (j c) -> p j c", j=G),
                    vm_sb[:, r * G:(r + 1) * G].unsqueeze(2)
                        .broadcast_to([CH, G, CELL]))
                nc.sync.dma_start(
                    y_ap[:, r * CELL:(r + 1) * CELL, :],
                    pat[:].unsqueeze(1).broadcast_to([CH, CELL, W]))
    nc.compile()
    return nc


def _get_kernels():
    if "pool" not in _CACHE:
        _CACHE["pool"] = _build_pool_kernel()
        _CACHE["paint"] = _build_paint_kernel()
    return _CACHE["pool"], _CACHE["paint"]


def _level_coords(g):
    c = (np.arange(g, dtype=np.float32) + 0.5) / g
    gy, gx = np.meshgrid(c, c, indexing="ij")
    centers = np.stack([gx, gy], axis=-1).reshape(-1, 2)
    sizes = np.full((g * g, 2), 1.0 / g, dtype=np.float32)
    return np.concatenate([centers, sizes], axis=-1).astype(np.float32)


def _middle(pooled8, ln_g, ln_b, w, b):
    """pooled8 [C, 8, 8] (one sample) -> (sparse_rows, coords_rows, valmap).

    Exactly mirrors the reference's per-level top-k / LayerNorm / Linear /
    coarse-to-fine paint, but on the 8x8 value map instead of dense HxW.
    """
    pyr = [None] * NUM_LEVELS
    pyr[NUM_LEVELS - 1] = pooled8
    for l in range(NUM_LEVELS - 2, -1, -1):
        g = 2 ** l
        p = pyr[l + 1].reshape(C, g, 2, g, 2)
        pyr[l] = p.max(axis=(2, 4))

    per_level = [None] * NUM_LEVELS
    for level in range(NUM_LEVELS - 1, -1, -1):
        g = 2 ** level
        N = g * g
        flat = pyr[level].reshape(C, N).T                    # [N, C]
        l2 = np.linalg.norm(flat.astype(np.float32), axis=1)
        if level < NUM_LEVELS - 1:
            parent = pyr[level + 1][:, ::2, ::2].reshape(C, N)
            pl2 = np.linalg.norm(parent, axis=0)
            imp = np.abs(l2 - pl2)
        else:
            imp = l2
        k = min(max(MIN_KEEPS, int(N * KEEP_RATIO)), N)
        idx = np.argsort(-imp, kind="stable")[:k]            # top_k order
        kf = flat[idx].astype(np.float32)                    # [k, C]
        mu = kf.mean(-1, keepdims=True)
        var = ((kf - mu) ** 2).mean(-1, keepdims=True)
        kfn = (kf - mu) / np.sqrt(var + EPS) * ln_g + ln_b
        kp = (kfn @ w + b).astype(np.float32)                # [k, C_OUT]
        kc = _level_coords(g)[idx]                           # [k, 4]
        per_level[level] = (kp, idx, kc)

    sparse_rows = np.concatenate([p[0] for p in per_level], axis=0)
    coords_rows = np.concatenate([p[2] for p in per_level], axis=0)

    valmap = np.zeros((C_OUT, G, G), np.float32)
    for level in range(NUM_LEVELS):
        kp, idx, _ = per_level[level]
        g = 2 ** level
        s = G // g
        for row, n in zip(kp, idx):
            y, x = divmod(int(n), g)
            valmap[:, y * s:(y + 1) * s, x * s:(x + 1) * s] = row[:, None, None]
    return sparse_rows, coords_rows, valmap


def kernel(x, ln_g, ln_b, w, b):
    x = np.ascontiguousarray(np.asarray(x, np.float32))
    ln_g = np.asarray(ln_g, np.float32)
    ln_b = np.asarray(ln_b, np.float32)
    w = np.asarray(w, np.float32)
    b = np.asarray(b, np.float32)

    nc_pool, nc_paint = _get_kernels()

    # --- device pass 1: grid max-pool, sharded (sample, channel-half) ---
    # x[s, h*128:(h+1)*128] is a contiguous view — zero-copy sharding.
    in_maps = [
        {"x": x[c // 2, (c % 2) * CH:(c % 2 + 1) * CH, :, :]}
        for c in range(8)
    ]
    r1 = bass_utils.run_bass_kernel_spmd(nc_pool, in_maps, core_ids=list(range(8)))
    pooled8 = np.empty((B, C, G, G), np.float32)
    for c in range(8):
        s, h = c // 2, c % 2
        pooled8[s, h * CH:(h + 1) * CH] = r1.results[c]["pooled"].reshape(CH, G, G)

    # --- host middle: top-k / LayerNorm / Linear / coords / value map ---
    sparse_seq = np.empty((B, 32, C_OUT), np.float32)
    all_coords = np.empty((B, 32, 4), np.float32)
    valmaps = np.empty((B, C_OUT, G, G), np.float32)
    for s in range(B):
        sparse_seq[s], all_coords[s], valmaps[s] = _middle(
            pooled8[s], ln_g, ln_b, w, b)

    # --- device pass 2: broadcast-paint the dense output ---
    in_maps2 = [
        {"vm": valmaps[c // 2, (c % 2) * CH:(c % 2 + 1) * CH].reshape(CH, G * G)}
        for c in range(8)
    ]
    r2 = bass_utils.run_bass_kernel_spmd(nc_paint, in_maps2, core_ids=list(range(8)))
    out = np.empty((B, C_OUT, H, W), np.float32)
    for c in range(8):
        s, h = c // 2, c % 2
        out[s, h * CH:(h + 1) * CH] = r2.results[c]["out"]

    sparsity = np.float32(sparse_seq.shape[1] / (H * W))
    return out, sparse_seq, all_coords, sparsity


# revision 5
# speedup vs baseline: 1.0468x; 1.0011x over previous
"""Trainium2 Bass kernel for nn_HIPABlock_42752104465010.

Structure of the computation (B=4, C=C_out=256, H=W=256, 4 pyramid levels):
  1. Grid max-pool pyramid: only the finest level (8x8 grid of 32x32 cells)
     needs the full input read; coarser levels are 2x2 maxes of it.
  2. Tiny middle: L2 norms, top-k selection, LayerNorm + Linear on <=32
     tokens per sample, coords. (KB-scale compute.)
  3. Dense output is piecewise-constant on the 8x8 grid of 32x32 blocks
     (level 1 keeps all 4 cells, finer levels overwrite), so it is a
     [C_out, 8, 8] value map broadcast up by 32x32 per sample.

Device plan (8 NeuronCores): core c = 2*s + h handles sample s = c//2 and
channel half h = c%2 (channels 128h : 128h+128, full image height). The
per-core slices of x/out are contiguous, so host-side sharding is zero-copy.
  Kernel A (pool):  x_half [128, 256, 256] -> pooled [128, 64]   (reads 32 MiB)
  Host:             top-k / LN / Linear / valmap  (numpy, ~KBs)
  Kernel B (paint): vm [128, 64] -> out_half [128, 256, 256]     (writes 32 MiB)
Total HBM traffic = 512 MiB across 8 cores == the memory roofline.
"""

import numpy as np

import concourse.tile as tile
from concourse import bacc, mybir
from concourse import bass_utils
from concourse._compat import get_trn_type

B = 4
C = 256
C_OUT = 256
H = 256
W = 256
NUM_LEVELS = 4
KEEP_RATIO = 0.3
MIN_KEEPS = 8
EPS = 1e-5

G = 2 ** (NUM_LEVELS - 1)          # 8  finest grid
CELL = H // G                      # 32 pixels per finest cell
CH = C // 2                        # 128 channels per core

F32 = mybir.dt.float32

_CACHE = {}


def _build_pool_kernel(sub=8, bufs=8):
    """x_half [128, 256, 256] -> pooled [128, 8*8] (max over 32x32 cells).

    Each grid-row is streamed as `sub` row-chunks; every chunk is reduced
    to its 8 per-cell partial maxes as soon as it lands (one 4D-AP
    reduce_max), then one tiny strided reduce combines the partials. This
    keeps the DVE tail after the last DMA to ~1 us.
    """
    nc = bacc.Bacc(get_trn_type() or "TRN2", target_bir_lowering=False,
                   debug=False, num_devices=8)
    x_ap = nc.dram_tensor("x", [CH, H, W], F32, kind="ExternalInput").ap()
    p_ap = nc.dram_tensor("pooled", [CH, G * G], F32, kind="ExternalOutput").ap()
    rows = CELL // sub

    with tile.TileContext(nc) as tc:
        with tc.tile_pool(name="big", bufs=bufs) as big, \
             tc.tile_pool(name="part", bufs=2) as part, \
             tc.tile_pool(name="acc", bufs=1) as acc:
            pooled_sb = acc.tile([CH, G * G], F32)
            for r in range(G):
                # last grid-row: split the final chunk 3+1 rows so the very
                # last reduce on the post-stream critical path is short
                if r == G - 1:
                    chunks = [(k * rows, rows) for k in range(sub - 1)]
                    chunks += [((sub - 1) * rows, rows - 1), (sub * rows - 1, 1)]
                else:
                    chunks = [(k * rows, rows) for k in range(sub)]
                partials = part.tile([CH, len(chunks) * G], F32, tag="partials")
                for ci, (st, n) in enumerate(chunks):
                    t = big.tile([CH, rows * W], F32, tag="big")
                    nc.sync.dma_start(
                        t[:, :n * W].rearrange("p (r c) -> p r c", r=n),
                        x_ap[:, r * CELL + st: r * CELL + st + n, :])
                    v = t[:, :n * W].rearrange("p (r j c) -> p j r c", r=n, j=G)
                    nc.vector.reduce_max(partials[:, ci * G:(ci + 1) * G], v,
                                         axis=mybir.AxisListType.XY)
                pv = partials[:].rearrange("p (k j) -> p j k", k=len(chunks))
                nc.vector.reduce_max(pooled_sb[:, r * G:(r + 1) * G], pv,
                                     axis=mybir.AxisListType.X)
            nc.sync.dma_start(p_ap[:, :], pooled_sb[:])
    nc.compile()
    return nc


def _build_paint_kernel():
    """vm [128, 8*8] -> out_half [128, 256, 256] (32x32 broadcast).

    Per grid-row: one stride-0 broadcast tensor_copy expands the 8 cell
    values into a one-row pattern, then one stride-0-source DMA writes all
    32 identical rows.
    """
    nc = bacc.Bacc(get_trn_type() or "TRN2", target_bir_lowering=False,
                   debug=False, num_devices=8)
    vm_ap = nc.dram_tensor("vm", [CH, G * G], F32, kind="ExternalInput").ap()
    y_ap = nc.dram_tensor("out", [CH, H, W], F32, kind="ExternalOutput").ap()

    with tile.TileContext(nc) as tc:
        with tc.tile_pool(name="vm", bufs=1) as vpool, \
             tc.tile_pool(name="pat", bufs=4) as ppool:
            vm_sb = vpool.tile([CH, G * G], F32)
            nc.sync.dma_start(vm_sb[:], vm_ap[:, :])
            for r in range(G):
                pat = ppool.tile([CH, W], F32)
                nc.vector.tensor_copy(
                    pat[:].rearrange("p (j c) -> p j c", j=G),
                    vm_sb[:, r * G:(r + 1) * G].unsqueeze(2)
                        .broadcast_to([CH, G, CELL]))
                nc.sync.dma_start(
                    y_ap[:, r * CELL:(r + 1) * CELL, :],
                    pat[:].unsqueeze(1).broadcast_to([CH, CELL, W]))
    nc.compile()
    return nc


def _get_kernels():
    if "pool" not in _CACHE:
        _CACHE["pool"] = _build_pool_kernel()
        _CACHE["paint"] = _build_paint_kernel()
    return _CACHE["pool"], _CACHE["paint"]


def _level_coords(g):
    c = (np.arange(g, dtype=np.float32) + 0.5) / g
    gy, gx = np.meshgrid(c, c, indexing="ij")
    centers = np.stack([gx, gy], axis=-1).reshape(-1, 2)
    sizes = np.full((g * g, 2), 1.0 / g, dtype=np.float32)
    return np.concatenate([centers, sizes], axis=-1).astype(np.float32)


def _middle(pooled8, ln_g, ln_b, w, b):
    """pooled8 [C, 8, 8] (one sample) -> (sparse_rows, coords_rows, valmap).

    Exactly mirrors the reference's per-level top-k / LayerNorm / Linear /
    coarse-to-fine paint, but on the 8x8 value map instead of dense HxW.
    """
    pyr = [None] * NUM_LEVELS
    pyr[NUM_LEVELS - 1] = pooled8
    for l in range(NUM_LEVELS - 2, -1, -1):
        g = 2 ** l
        p = pyr[l + 1].reshape(C, g, 2, g, 2)
        pyr[l] = p.max(axis=(2, 4))

    per_level = [None] * NUM_LEVELS
    for level in range(NUM_LEVELS - 1, -1, -1):
        g = 2 ** level
        N = g * g
        flat = pyr[level].reshape(C, N).T                    # [N, C]
        l2 = np.linalg.norm(flat.astype(np.float32), axis=1)
        if level < NUM_LEVELS - 1:
            parent = pyr[level + 1][:, ::2, ::2].reshape(C, N)
            pl2 = np.linalg.norm(parent, axis=0)
            imp = np.abs(l2 - pl2)
        else:
            imp = l2
        k = min(max(MIN_KEEPS, int(N * KEEP_RATIO)), N)
        idx = np.argsort(-imp, kind="stable")[:k]            # top_k order
        kf = flat[idx].astype(np.float32)                    # [k, C]
        mu = kf.mean(-1, keepdims=True)
        var = ((kf - mu) ** 2).mean(-1, keepdims=True)
        kfn = (kf - mu) / np.sqrt(var + EPS) * ln_g + ln_b
        kp = (kfn @ w + b).astype(np.float32)                # [k, C_OUT]
        kc = _level_coords(g)[idx]                           # [k, 4]
        per_level[level] = (kp, idx, kc)

    sparse_rows = np.concatenate([p[0] for p in per_level], axis=0)
    coords_rows = np.concatenate([p[2] for p in per_level], axis=0)

    valmap = np.zeros((C_OUT, G, G), np.float32)
    for level in range(NUM_LEVELS):
        kp, idx, _ = per_level[level]
        g = 2 ** level
        s = G // g
        for row, n in zip(kp, idx):
            y, x = divmod(int(n), g)
            valmap[:, y * s:(y + 1) * s, x * s:(x + 1) * s] = row[:, None, None]
    return sparse_rows, coords_rows, valmap


def kernel(x, ln_g, ln_b, w, b):
    x = np.ascontiguousarray(np.asarray(x, np.float32))
    ln_g = np.asarray(ln_g, np.float32)
    ln_b = np.asarray(ln_b, np.float32)
    w = np.asarray(w, np.float32)
    b = np.asarray(b, np.float32)

    nc_pool, nc_paint = _get_kernels()

    # --- device pass 1: grid max-pool, sharded (sample, channel-half) ---
    # x[s, h*128:(h+1)*128] is a contiguous view — zero-copy sharding.
    in_maps = [
        {"x": x[c // 2, (c % 2) * CH:(c % 2 + 1) * CH, :, :]}
        for c in range(8)
    ]
    r1 = bass_utils.run_bass_kernel_spmd(nc_pool, in_maps, core_ids=list(range(8)))
    pooled8 = np.empty((B, C, G, G), np.float32)
    for c in range(8):
        s, h = c // 2, c % 2
        pooled8[s, h * CH:(h + 1) * CH] = r1.results[c]["pooled"].reshape(CH, G, G)

    # --- host middle: top-k / LayerNorm / Linear / coords / value map ---
    sparse_seq = np.empty((B, 32, C_OUT), np.float32)
    all_coords = np.empty((B, 32, 4), np.float32)
    valmaps = np.empty((B, C_OUT, G, G), np.float32)
    for s in range(B):
        sparse_seq[s], all_coords[s], valmaps[s] = _middle(
            pooled8[s], ln_g, ln_b, w, b)

    # --- device pass 2: broadcast-paint the dense output ---
    in_maps2 = [
        {"vm": valmaps[c // 2, (c % 2) * CH:(c % 2 + 1) * CH].reshape(CH, G * G)}
        for c in range(8)
    ]
    r2 = bass_utils.run_bass_kernel_spmd(nc_paint, in_maps2, core_ids=list(range(8)))
    out = np.empty((B, C_OUT, H, W), np.float32)
    for c in range(8):
        s, h = c // 2, c % 2
        out[s, h * CH:(h + 1) * CH] = r2.results[c]["out"]

    sparsity = np.float32(sparse_seq.shape[1] / (H * W))
    return out, sparse_seq, all_coords, sparsity
